# revision 10
# baseline (speedup 1.0000x reference)
"""Trainium2 Bass kernel for the convolutional LUT-tree layer.

Math: each tree node computes out = sum_{p,q,r} sigmoid(lut)[p,q,r] *
Wa_p * Wb_q * Wc_r where Wa/Wb/Wc are soft one-hot pair weights of the six
gathered inputs (a,b | c,d | e,f).  On host we Moebius-transform the LUT into
monomial-basis coefficients C[u,v,w] over bases Ya=[1,a,b,ab], Yb=[1,c,d,cd],
Yc=[1,e,f,ef], so the device evaluates

    out = sum_u Ya_u * (sum_v Yb_v * (C[u,v,0] + e*C[u,v,1] + f*C[u,v,2]
                                      + ef*C[u,v,3]))

with nodes on partitions (per-node coefficients = per-partition scalars) and
the 1024 patch samples on the free dimension.  Gathers (patch features ->
node inputs, previous layer -> next layer inputs) are one-hot matmuls on the
tensor engine.  The 128 trees are sharded 16 per NeuronCore.
"""

import os
import numpy as np

import concourse.bass as bass
import concourse.bacc as bacc
import concourse.mybir as mybir
from concourse.tile import TileContext
from concourse.bass_utils import run_bass_kernel_spmd

F32 = mybir.dt.float32
MULT = mybir.AluOpType.mult
ADD = mybir.AluOpType.add
IDENT = mybir.ActivationFunctionType.Identity

# static problem dims
B, IH, IW = 64, 8, 8
KH = KW = 5
OH, OW = IH - KH + 1, IW - KW + 1            # 4, 4
N = B * OH * OW                              # 1024 patch samples
NF = KH * KW                                 # 25 patch features
T = 128                                      # trees
NCORES = 8
TL = T // NCORES                             # 16 trees per core
M0, M1, M2 = 36, 6, 1
N0, N1, N2 = TL * M0, TL * M1, TL * M2       # 576, 96, 16 nodes per core
CH0 = [128, 128, 128, 128, 64]               # layer-0 node chunks
NB = 512                                     # psum free-dim block (fp32 bank)

# Wa_p = sum_u MOB[p, u] * [1, a, b, ab]_u
MOB = np.array(
    [[1, -1, -1, 1],
     [0, 0, 1, -1],
     [0, 1, 0, -1],
     [0, 0, 0, 1]], dtype=np.float64)


def _monomial_coefs(lut):
    """lut [nodes, 64] raw params -> C [nodes, 64] monomial coefs, k=(u,v,w)."""
    p = 1.0 / (1.0 + np.exp(-lut.astype(np.float64)))
    p = p.reshape(-1, 4, 4, 4)
    c = np.einsum('npqr,pu,qv,rw->nuvw', p, MOB, MOB, MOB)
    return np.ascontiguousarray(c.reshape(-1, 64)).astype(np.float32)


def _build_program():
    nc = bacc.Bacc("TRN2", target_bir_lowering=False, debug=True)
    featT_d = nc.dram_tensor("featT", [NF, N], F32, kind="ExternalInput")
    oneh0_d = nc.dram_tensor("oneh0", [NF, 5 * 6 * 128], F32, kind="ExternalInput")
    coef0_d = nc.dram_tensor("coef0", [128, 5 * 64], F32, kind="ExternalInput")
    oneh1_d = nc.dram_tensor("oneh1", [5 * 128, 6 * N1], F32, kind="ExternalInput")
    coef1_d = nc.dram_tensor("coef1", [N1, 64], F32, kind="ExternalInput")
    oneh2_d = nc.dram_tensor("oneh2", [N1, 6 * N2], F32, kind="ExternalInput")
    coef2_d = nc.dram_tensor("coef2", [N2, 64], F32, kind="ExternalInput")
    out_d = nc.dram_tensor("out", [N2, N], F32, kind="ExternalOutput")

    with TileContext(nc) as tc:
        with (
            tc.tile_pool(name="const", bufs=1) as cp,
            tc.tile_pool(name="plane", bufs=2) as gp,
            tc.tile_pool(name="ev", bufs=2) as ev,
            tc.tile_pool(name="psum", bufs=8, space="PSUM") as pp,
        ):
            # ---- constants into SBUF
            featT = cp.tile([NF, N], F32, tag="featT", name="featT")
            nc.sync.dma_start(out=featT[:, :], in_=featT_d[:, :])
            oneh0 = cp.tile([NF, 5 * 6 * 128], F32, tag="oneh0", name="oneh0")
            nc.sync.dma_start(out=oneh0[:, :], in_=oneh0_d[:, :])
            coef0 = cp.tile([128, 5 * 64], F32, tag="coef0", name="coef0")
            nc.sync.dma_start(out=coef0[:, :], in_=coef0_d[:, :])
            oneh1 = [cp.tile([128, 6 * N1], F32, tag=f"oneh1_{c}", name=f"oneh1_{c}") for c in range(5)]
            for c in range(5):
                nc.sync.dma_start(out=oneh1[c][:, :],
                                  in_=oneh1_d[c * 128:(c + 1) * 128, :])
            coef1 = cp.tile([N1, 64], F32, tag="coef1", name="coef1")
            nc.sync.dma_start(out=coef1[:, :], in_=coef1_d[:, :])
            oneh2 = cp.tile([N1, 6 * N2], F32, tag="oneh2", name="oneh2")
            nc.sync.dma_start(out=oneh2[:, :], in_=oneh2_d[:, :])
            coef2 = cp.tile([N2, 64], F32, tag="coef2", name="coef2")
            nc.sync.dma_start(out=coef2[:, :], in_=coef2_d[:, :])

            h0 = [cp.tile([128, N], F32, tag=f"h0_{c}", name=f"h0_{c}") for c in range(5)]
            h1 = cp.tile([N1, N], F32, tag="h1", name="h1")
            h2 = cp.tile([N2, N], F32, tag="h2", name="h2")

            toggle = [0]

            def eng():
                toggle[0] ^= 1
                return nc.vector if toggle[0] else nc.gpsimd

            def eval_chunk(nnode, planes, coefT, coff, out_ap):
                """planes: 6 SBUF [>=nnode, N] gathered inputs a,b,c,d,e,f.
                coefT[:, coff+k] holds C_k, k=(u*4+v)*4+w.  Writes out_ap."""
                a, b, c, d, e, f = [pl[:nnode, :] for pl in planes]

                def co(k):
                    return coefT[:nnode, coff + k:coff + k + 1]

                ab = ev.tile([128, N], F32, tag="prod_ab", name="prod_ab")[:nnode, :]
                cd = ev.tile([128, N], F32, tag="prod_cd", name="prod_cd")[:nnode, :]
                ef = ev.tile([128, N], F32, tag="prod_ef", name="prod_ef")[:nnode, :]
                eng().tensor_tensor(ab, a, b, MULT)
                eng().tensor_tensor(cd, c, d, MULT)
                eng().tensor_tensor(ef, e, f, MULT)

                tu = []
                for u in range(4):
                    svs = []
                    for v in range(4):
                        k0 = (u * 4 + v) * 4
                        s = ev.tile([128, N], F32, tag="S", name="S", bufs=4)[:nnode, :]
                        # S = C0 + e*C1 + f*C2 + ef*C3
                        nc.scalar.activation(s, e, IDENT,
                                             bias=co(k0 + 0), scale=co(k0 + 1))
                        nc.vector.scalar_tensor_tensor(s, f, co(k0 + 2), s, MULT, ADD)
                        nc.vector.scalar_tensor_tensor(s, ef, co(k0 + 3), s, MULT, ADD)
                        svs.append(s)
                    # T_u = S0 + c*S1 + d*S2 + cd*S3
                    t = ev.tile([128, N], F32, tag="T", name="T", bufs=4)[:nnode, :]
                    tm = ev.tile([128, N], F32, tag="tmp", name="tmp")[:nnode, :]
                    eng().tensor_tensor(t, c, svs[1], MULT)
                    eng().tensor_tensor(t, t, svs[0], ADD)
                    eng().tensor_tensor(tm, d, svs[2], MULT)
                    eng().tensor_tensor(t, t, tm, ADD)
                    eng().tensor_tensor(tm, cd, svs[3], MULT)
                    eng().tensor_tensor(t, t, tm, ADD)
                    tu.append(t)
                # out = T0 + a*T1 + b*T2 + ab*T3
                tm = ev.tile([128, N], F32, tag="tmp2", name="tmp2", bufs=1)[:nnode, :]
                acc = ev.tile([128, N], F32, tag="acc", name="acc", bufs=1)[:nnode, :]
                eng().tensor_tensor(acc, a, tu[1], MULT)
                eng().tensor_tensor(acc, acc, tu[0], ADD)
                eng().tensor_tensor(tm, b, tu[2], MULT)
                eng().tensor_tensor(acc, acc, tm, ADD)
                eng().tensor_tensor(tm, ab, tu[3], MULT)
                eng().tensor_tensor(out_ap, acc, tm, ADD)

            # ---------------- layer 0 ----------------
            for c in range(5):
                ch = CH0[c]
                planes = [gp.tile([128, N], F32, tag=f"g{j}", name=f"g{j}") for j in range(6)]
                for j in range(6):
                    col0 = (c * 6 + j) * 128
                    for nb in range(N // NB):
                        ps = pp.tile([128, NB], F32, tag="ps", name="ps")
                        nc.tensor.matmul(
                            out=ps[:ch, :],
                            lhsT=oneh0[:, col0:col0 + ch],
                            rhs=featT[:, nb * NB:(nb + 1) * NB],
                            start=True, stop=True)
                        nc.scalar.copy(
                            out=planes[j][:ch, nb * NB:(nb + 1) * NB],
                            in_=ps[:ch, :])
                eval_chunk(ch, planes, coef0, c * 64, h0[c][:ch, :])

            # ---------------- layer 1 ----------------
            planes1 = [gp.tile([128, N], F32, tag=f"g{j}", name=f"g1_{j}") for j in range(6)]
            for j in range(6):
                for nb in range(N // NB):
                    ps = pp.tile([128, NB], F32, tag="ps", name="ps1")
                    for c in range(5):
                        nc.tensor.matmul(
                            out=ps[:N1, :],
                            lhsT=oneh1[c][:CH0[c], j * N1:(j + 1) * N1],
                            rhs=h0[c][:CH0[c], nb * NB:(nb + 1) * NB],
                            start=(c == 0), stop=(c == 4))
                    nc.scalar.copy(
                        out=planes1[j][:N1, nb * NB:(nb + 1) * NB],
                        in_=ps[:N1, :])
            eval_chunk(N1, planes1, coef1, 0, h1[:, :])

            # ---------------- layer 2 ----------------
            planes2 = [gp.tile([128, N], F32, tag=f"g{j}", name=f"g2_{j}") for j in range(6)]
            for nb in range(N // NB):
                for j in range(6):
                    ps = pp.tile([128, NB], F32, tag="ps", name="ps2")
                    nc.tensor.matmul(
                        out=ps[:N2, :],
                        lhsT=oneh2[:N1, j * N2:(j + 1) * N2],
                        rhs=h1[:N1, nb * NB:(nb + 1) * NB],
                        start=True, stop=True)
                    nc.scalar.copy(
                        out=planes2[j][:N2, nb * NB:(nb + 1) * NB],
                        in_=ps[:N2, :])
            eval_chunk(N2, planes2, coef2, 0, h2[:, :])

            nc.sync.dma_start(out=out_d[:, :], in_=h2[:N2, :])
    nc.finalize()
    return nc


_NC_CACHE = {}


def _get_program():
    if "nc" not in _NC_CACHE:
        _NC_CACHE["nc"] = _build_program()
    return _NC_CACHE["nc"]


def _im2col(x):
    """x [B,1,IH,IW] -> featT [25, N] with feature-major partitions."""
    x = np.asarray(x, dtype=np.float32).reshape(B, IH, IW)
    featT = np.empty((NF, N), dtype=np.float32)
    for ki in range(KH):
        for kj in range(KW):
            patch = x[:, ki:ki + OH, kj:kj + OW]          # [B, 4, 4]
            featT[ki * KW + kj, :] = patch.reshape(-1)
    return featT


def _core_inputs(cid, featT, idx0, lut0, idx1, lut1, idx2, lut2):
    ts = slice(cid * TL, (cid + 1) * TL)
    idx0c = idx0[ts].reshape(N0, 6)
    oneh0 = np.zeros((NF, 5, 6, 128), dtype=np.float32)
    for c in range(5):
        lo, hi = c * 128, min((c + 1) * 128, N0)
        for j in range(6):
            oneh0[idx0c[lo:hi, j], c, j, np.arange(hi - lo)] = 1.0
    coef0_full = _monomial_coefs(lut0[ts].reshape(N0, 64))
    coef0 = np.zeros((128, 5, 64), dtype=np.float32)
    for c in range(5):
        lo, hi = c * 128, min((c + 1) * 128, N0)
        coef0[:hi - lo, c, :] = coef0_full[lo:hi]

    # layer-1 gather: h0 row = t*36 + idx1
    rows1 = np.arange(TL)[:, None, None] * M0 + idx1[ts]      # [16, 6, 6]
    oneh1 = np.zeros((5 * 128, 6, N1), dtype=np.float32)
    for t in range(TL):
        for m in range(M1):
            for j in range(6):
                oneh1[rows1[t, m, j], j, t * M1 + m] = 1.0
    coef1 = _monomial_coefs(lut1[ts].reshape(N1, 64))

    # layer-2 gather: h1 row = t*6 + idx2
    oneh2 = np.zeros((N1, 6, N2), dtype=np.float32)
    for t in range(TL):
        for j in range(6):
            oneh2[t * M1 + idx2[ts][t, 0, j], j, t] = 1.0
    coef2 = _monomial_coefs(lut2[ts].reshape(N2, 64))

    return {
        "featT": featT,
        "oneh0": oneh0.reshape(NF, 5 * 6 * 128),
        "coef0": coef0.reshape(128, 5 * 64),
        "oneh1": oneh1.reshape(5 * 128, 6 * N1),
        "coef1": coef1,
        "oneh2": oneh2.reshape(N1, 6 * N2),
        "coef2": coef2,
    }


def kernel(x, idx0, lut0, idx1, lut1, idx2, lut2):
    x = np.asarray(x, dtype=np.float32)
    idx0 = np.asarray(idx0)
    idx1 = np.asarray(idx1)
    idx2 = np.asarray(idx2)
    lut0 = np.asarray(lut0, dtype=np.float32)
    lut1 = np.asarray(lut1, dtype=np.float32)
    lut2 = np.asarray(lut2, dtype=np.float32)

    featT = _im2col(x)
    in_maps = [
        _core_inputs(cid, featT, idx0, lut0, idx1, lut1, idx2, lut2)
        for cid in range(NCORES)
    ]
    nc = _get_program()
    trace = os.environ.get("BASS_KERNEL_TRACE", "0") == "1"
    try:
        res = run_bass_kernel_spmd(nc, in_maps, core_ids=list(range(NCORES)),
                                   trace=trace)
    except Exception:
        if not trace:
            raise
        res = run_bass_kernel_spmd(nc, in_maps, core_ids=list(range(NCORES)),
                                   trace=False)
    if trace:
        _NC_CACHE["last_results"] = res
    full = np.concatenate([res.results[cid]["out"] for cid in range(NCORES)],
                          axis=0)                     # [128, 1024]
    out = full.reshape(T, B, OH * OW).transpose(1, 0, 2).reshape(B, T, OH, OW)
    return np.ascontiguousarray(out.astype(np.float32))
